# revision 1
# baseline (speedup 1.0000x reference)
"""Trainium2 Bass kernel for PointCloudAligner (chamfer-style K=1 NN loss).

loss = mean_i min_j || exp(s)*src_i + t - tgt_j ||^2  + 0.1*relu(-s)

Strategy (8 NeuronCores, SPMD):
  - Shard source points (rows of the 16384x16384 distance matrix) across the
    8 cores: 2048 source rows per core. Targets are broadcast to all cores.
  - Per core, compute d2[i,j] via TensorE matmul with an augmented contraction:
        d2[i,j] = sq_src_i + sq_tgt_j - 2*tp_i . tgt_j
    All inputs are decomposed into 3-term bf16 sums (hi/mid/lo) so the PE runs
    at bf16 speed (1 cycle/column) while retaining fp32 accuracy: the cross
    products of the terms are stacked into the (otherwise tiny) contraction
    dim.  Coord pairs kept: (h,h),(h,m),(m,h),(h,l),(l,h),(m,m) -> residual
    ~2^-35, i.e. exact at fp32 level.
        K = 18 (coord pairs) + 3 (sq_src 3-term) + 3 (sq_tgt 3-term) = 24
  - VectorE reduces min over each [128, 2048] PSUM superchunk; per-row minima
    are collected and DMA'd out; the final mean is a host-side gather.
"""

import numpy as np

N_CORES = 8
N = 16384  # source points
M = 16384  # target points
N_LOC = N // N_CORES  # 2048 source rows per core
P = 128  # partitions
I_TILES = N_LOC // P  # 16 row tiles per core
JC = 512  # moving free dim per matmul (one PSUM bank, fp32)
SUPER = 2048  # psum superchunk (4 banks)
N_SUPER = M // SUPER  # 8
K = 24  # augmented contraction dim

_CACHE = {}


def _bf16_split(x, n_terms):
    """Decompose fp32 array into n bf16 terms summing to ~x."""
    import ml_dtypes

    bf16 = ml_dtypes.bfloat16
    terms = []
    r = np.asarray(x, dtype=np.float32)
    for _ in range(n_terms):
        t = r.astype(bf16)
        terms.append(t)
        r = (r - t.astype(np.float32)).astype(np.float32)
    return terms


def _build_program():
    import concourse.bass as bass
    import concourse.tile as tile
    from concourse import mybir

    nc = bass.Bass("TRN2", target_bir_lowering=False, debug=False)
    lhs_d = nc.dram_tensor("lhs", [K, N_LOC], mybir.dt.bfloat16, kind="ExternalInput")
    rhs_d = nc.dram_tensor("rhs", [K, M], mybir.dt.bfloat16, kind="ExternalInput")
    out_d = nc.dram_tensor("mins", [P, I_TILES], mybir.dt.float32, kind="ExternalOutput")

    with tile.TileContext(nc) as tc:
        with (
            tc.tile_pool(name="singles", bufs=1) as singles,
            tc.tile_pool(name="psum", bufs=2, space="PSUM") as psum_pool,
            tc.tile_pool(name="work", bufs=4) as work,
        ):
            lhs_s = singles.tile([K, N_LOC], mybir.dt.bfloat16)
            rhs_s = singles.tile([K, M], mybir.dt.bfloat16)
            nc.sync.dma_start(out=lhs_s, in_=lhs_d[:, :])
            nc.sync.dma_start(out=rhs_s, in_=rhs_d[:, :])
            mins_sb = singles.tile([P, I_TILES], mybir.dt.float32)

            for t in range(I_TILES):
                part = work.tile([P, N_SUPER], mybir.dt.float32, tag="part")
                for s in range(N_SUPER):
                    ps = psum_pool.tile([P, SUPER], mybir.dt.float32, tag="ps")
                    for q in range(SUPER // JC):
                        j0 = s * SUPER + q * JC
                        nc.tensor.matmul(
                            ps[:, q * JC : (q + 1) * JC],
                            lhs_s[:, t * P : (t + 1) * P],
                            rhs_s[:, j0 : j0 + JC],
                            start=True,
                            stop=True,
                        )
                    nc.vector.tensor_reduce(
                        part[:, s : s + 1],
                        ps[:, :],
                        axis=mybir.AxisListType.X,
                        op=mybir.AluOpType.min,
                    )
                nc.vector.tensor_reduce(
                    mins_sb[:, t : t + 1],
                    part[:, :],
                    axis=mybir.AxisListType.X,
                    op=mybir.AluOpType.min,
                )
            nc.sync.dma_start(out=out_d[:, :], in_=mins_sb)

    _strip_redundant_mm_self_waits(nc, mybir)
    return nc


def _strip_redundant_mm_self_waits(nc, mybir):
    """walrus can encode only a limited number of sync waits per instruction
    (1 for Matmult, ~4 for NOP-class). Tile's wait emission is per-engine
    minimal but NOT transitively minimal, so instructions often carry waits
    already implied by their other waits. Compute each semaphore tick's
    transitive closure and drop implied waits.

    Model: completion of instruction I implies (a) completion of all earlier
    instructions on I's engine (in-order engines; per-queue FIFO for DMA),
    (b) satisfaction of all waits I carried. A DMA's *completion tick* (the
    HWDGE sem bump, +16) implies the waits carried by the dma_start and all
    earlier completions on the same queue."""
    import bisect

    # Gather instructions in scheduled order with waits and sem updates.
    events = []  # (stream_key, waits[(sem,val)], updates[(sem,val_after)])
    sem_counts = {}
    inst_entries = []
    for f in nc.m.functions:
        for b in f.blocks:
            for inst in b.instructions:
                si = inst.sync_info
                waits = []
                updates = []
                if si and si.on_wait:
                    for w in si.on_wait:
                        if w.wait_value is None or str(w.wait_mode) != "sem-ge-imm":
                            continue  # register/eq waits: not reasoned about
                        waits.append((str(w.ant_name), int(w.wait_value)))
                if si and si.on_update:
                    for u in si.on_update:
                        s = str(u.ant_name)
                        inc = 16 if s.startswith("DMA") else 1
                        sem_counts[s] = sem_counts.get(s, 0) + inc
                        updates.append((s, sem_counts[s]))
                # Completion-stream key: compute engines complete in order;
                # DMAs complete FIFO per HW queue (identified by their sem).
                dma_sems = [s for s, _ in updates if s.startswith("DMA")]
                key = dma_sems[0] if dma_sems else f"eng:{inst.engine}"
                events.append((key, waits, updates))
                inst_entries.append(inst)

    # closure[(sem, tick)] = {sem2: value known reached when that tick fires}
    closure = {}
    ticks = {}  # sem -> sorted list of tick values
    stream_state = {}

    def tick_closure(s, v):
        """Closure of the earliest tick >= v on sem s (what a satisfied
        wait (s >= v) guarantees)."""
        tl = ticks.get(s)
        if not tl:
            return None
        i = bisect.bisect_left(tl, v)
        if i == len(tl):
            return None
        return closure.get((s, tl[i]))

    for key, waits, updates in events:
        st = dict(stream_state.get(key, {}))
        if waits:
            for s, v in waits:
                st[s] = max(st.get(s, 0), v)
                impl = tick_closure(s, v)
                if impl:
                    for s2, v2 in impl.items():
                        st[s2] = max(st.get(s2, 0), v2)
        stream_state[key] = st
        for s, v in updates:
            d = dict(st)
            d[s] = v
            closure[(s, v)] = d
            ticks.setdefault(s, []).append(v)  # built in increasing order

    for inst in inst_entries:
        si = inst.sync_info
        if not si or not si.on_wait or len(si.on_wait) < 2:
            continue
        if any(
            w.wait_value is None or str(w.wait_mode) != "sem-ge-imm"
            for w in si.on_wait
        ):
            continue
        # Self-engine waits are redundant on serially-executing engines
        # (strict-FIFO, one op at a time): program order already guarantees
        # the previous op on this engine completed. Tile emits them for
        # same-engine PSUM/buffer-reuse tracking; drop when over budget.
        eng_prefix = str(inst.engine).split(".")[-1] + "_"
        keep = [w for w in si.on_wait if not str(w.ant_name).startswith(eng_prefix)]
        if not keep:
            keep = list(si.on_wait)[-1:]
        if len(keep) >= 2:
            pass  # fall through to transitive pruning below
        if len(keep) < len(si.on_wait):
            inst.sync_info = mybir.SyncInfo(
                on_wait=list(keep), on_update=list(si.on_update or [])
            )
            si = inst.sync_info
        if len(si.on_wait) < 2:
            continue
        keep = list(si.on_wait)
        changed = True
        while changed and len(keep) > 1:
            changed = False
            for i in range(len(keep)):
                s, v = str(keep[i].ant_name), int(keep[i].wait_value)
                for j in range(len(keep)):
                    if j == i:
                        continue
                    impl = tick_closure(
                        str(keep[j].ant_name), int(keep[j].wait_value)
                    )
                    if impl and impl.get(s, 0) >= v:
                        keep.pop(i)
                        changed = True
                        break
                if changed:
                    break
        if len(keep) < len(si.on_wait):
            inst.sync_info = mybir.SyncInfo(
                on_wait=keep, on_update=list(si.on_update or [])
            )


def _prepare_inputs(source_points, target_points, scale, translation):
    """Host-side affine transform + hi/lo bf16 augmentation (tiny: O(N*3))."""
    src = np.asarray(source_points, dtype=np.float32)
    tgt = np.asarray(target_points, dtype=np.float32)
    s = np.exp(np.float32(scale.reshape(-1)[0]))
    tr = np.asarray(translation, dtype=np.float32).reshape(1, 3)
    tp = (src * s + tr).astype(np.float32)  # [N,3]

    sq_src = np.sum(tp * tp, axis=1, dtype=np.float32)  # [N]
    sq_tgt = np.sum(tgt * tgt, axis=1, dtype=np.float32)  # [M]
    m2t = (-2.0 * tgt).astype(np.float32)  # [M,3]

    ah, am, al = _bf16_split(tp, 3)  # source coord terms, [N,3] bf16 each
    bh, bm, bl = _bf16_split(m2t, 3)  # target coord terms (-2*tgt)
    sqs = _bf16_split(sq_src, 3)  # 3 x [N]
    sqt = _bf16_split(sq_tgt, 3)

    import ml_dtypes

    bf16 = ml_dtypes.bfloat16
    ones_n = np.ones(N, dtype=bf16)
    ones_m = np.ones(M, dtype=bf16)

    # lhs rows pair with rhs rows (contraction): coordinate term pairs
    # (h,h),(h,m),(m,h),(h,l),(l,h),(m,m) x 3 dims, then sq rows.
    coord_pairs = [(ah, bh), (ah, bm), (am, bh), (ah, bl), (al, bh), (am, bm)]
    lhs_rows = []
    rhs_rows = []
    for a, b in coord_pairs:
        for d in range(3):
            lhs_rows.append(a[:, d])
            rhs_rows.append(b[:, d])
    lhs_rows += [sqs[0], sqs[1], sqs[2], ones_n, ones_n, ones_n]
    rhs_rows += [ones_m, ones_m, ones_m, sqt[0], sqt[1], sqt[2]]
    lhs_full = np.stack(lhs_rows, axis=0)  # [K, N] bf16
    rhs_full = np.stack(rhs_rows, axis=0)  # [K, M] bf16

    in_maps = []
    for c in range(N_CORES):
        lhs_c = np.ascontiguousarray(lhs_full[:, c * N_LOC : (c + 1) * N_LOC])
        in_maps.append({"lhs": lhs_c, "rhs": np.ascontiguousarray(rhs_full)})
    return in_maps


def run_on_device(in_maps, trace=False, **kw):
    from concourse.bass_utils import run_bass_kernel_spmd

    if "nc" not in _CACHE:
        _CACHE["nc"] = _build_program()
    nc = _CACHE["nc"]
    return run_bass_kernel_spmd(nc, in_maps, list(range(N_CORES)), trace=trace, **kw)


def kernel(source_points, target_points, scale, translation):
    in_maps = _prepare_inputs(source_points, target_points, scale, translation)
    res = run_on_device(in_maps)
    mins = np.concatenate([r["mins"].reshape(-1) for r in res.results])
    assert mins.size == N
    sc = np.float32(np.asarray(scale, dtype=np.float32).reshape(-1)[0])
    loss = np.float32(np.mean(mins, dtype=np.float64)) + np.float32(0.1) * max(
        np.float32(0.0), -sc
    )
    return np.float32(loss)

